# revision 14
# baseline (speedup 1.0000x reference)
"""GCN layer kernel for 8 trn2 NeuronCores (SPMD, single launch).

Math:  out = D^-1/2 (A+I) D^-1/2 X W^T + b
Identity: the dense layer commutes with the diagonal scalings:
    out = D^-1/2 (A+I) D^-1/2 (X W^T) + b
so U = X@W^T (tiny) is computed on-chip, then one big matmul A @ (d^-1/2*U).

Sharding: row-shard A (1024 rows/core).  Each core receives its strip
PRE-TRANSPOSED to A^T layout and cast to bf16 on the host (16.8MB/core,
half the fp32 bytes; the transposed layout is what the PE contraction
needs, eliminating on-device PE transposes).  The strip is shipped in
SBUF order ([128, 65536], half-i major) so DMA runs as 16 x 1MB
transfers with 8KB contiguous per partition line.

The per-core j-axis (contraction axis) is ROTATED so each core's own
1024 nodes occupy j-tile slots 0..7: slot s holds global tile
(c*8+s)%64.  With this, "local vs remote" slot sets are uniform across
the shared SPMD program (slot s covers AG-half s%8//4 on every core).

Degree exchange is a SPLIT AllGather: rows i<512 of every core finish
DMA first, their row sums AllGather (AG1) while the second half still
streams; AG2 goes out at stream end and overlaps with the first half
of the big matmul.  Between the two matmul halves, dinv-gated filler
matmuls keep the PE HAM clock at 8/8.

Per core:
  - stream A^T strip; row sums accumulate on PE via a ones-column
    stationary matmul (one [1,512] PSUM chain per i-half).
  - U = X@W^T for all 8192 nodes (64 small matmuls, overlaps DMA).
  - AG1/AG2 local row sums -> full degree; per-core permutation
    matmuls map gathered degrees to rotated slot order; rsqrt+Newton.
  - Z^T accumulates in 2 PSUM banks: for each slot s,
    zacc_h += Y_s[j,f].T @ At_s[j,i_h]   (Y stationary, N=512 moving)
  - epilogue: 8 PE transposes of Z^T -> natural Z, + Y_local,
    * d_i^-1/2, + b, DMA out.
"""

import numpy as np
import ml_dtypes

N = 8192          # nodes
F = 128           # in/out feature dim
NCORES = 8
SR = N // NCORES  # strip rows per core = 1024
P = 128           # partitions / tile edge
IT = SR // P      # 8 row tiles per strip
JT = N // P       # 64 contraction tiles
H = 512           # psum-bank half of a 1024-wide row
NG = 8            # slots per DMA group (1MB transfers)
FILLN = 56        # PE keep-warm fillers between matmul halves

# slots whose global tile lands in AG-half 1 vs 2 (uniform across cores:
# slot s holds global tile (c*8+s)%64, whose i-half is (s%8)//4)
S1 = [s for s in range(JT) if s % 8 < 4]
S2 = [s for s in range(JT) if s % 8 >= 4]

_CACHE = {}


def _build_nc():
    import concourse.mybir as mybir
    from concourse import bass
    from concourse.tile import TileContext

    f32 = mybir.dt.float32
    bf16 = mybir.dt.bfloat16
    AF = mybir.ActivationFunctionType

    nc = bass.Bass(num_devices=NCORES)

    # at layout: [:, 0:32768] half0 = cols s*512+i (i<512) of slot s;
    #            [:, 32768:]  half1 = cols 32768+s*512+(i-512)
    A_t = nc.declare_dram_parameter("at", [P, JT * SR], bf16, False)
    Xt = nc.declare_dram_parameter("xt", [P, N], bf16, False)      # rotated X^T
    Wt = nc.declare_dram_parameter("wt", [P, F], bf16, False)      # W^T
    Bb = nc.declare_dram_parameter("bb", [P, F], f32, False)       # bias bcast
    Idn = nc.declare_dram_parameter("ident", [P, P], f32, False)
    On1 = nc.declare_dram_parameter("ones1", [P, 1], bf16, False)
    Pm1 = nc.declare_dram_parameter("perm1", [JT // 2, JT // 2], f32, False)
    Pm2 = nc.declare_dram_parameter("perm2", [JT // 2, JT // 2], f32, False)
    out = nc.declare_dram_parameter("out", [SR, F], f32, True)

    dumL = nc.dram_tensor("dum_loc", [1, 2], f32)
    dumA = nc.dram_tensor("dum_all", [NCORES, 2], f32, addr_space="Shared")
    degL1 = nc.dram_tensor("deg_loc1", [1, H], f32)
    degL2 = nc.dram_tensor("deg_loc2", [1, H], f32)
    degA1 = nc.dram_tensor("deg_all1", [JT // 2, P], f32, addr_space="Shared")
    degA2 = nc.dram_tensor("deg_all2", [JT // 2, P], f32, addr_space="Shared")

    HB = JT * H  # 32768, half-block width in the at layout

    with TileContext(nc) as tc:
        with tc.tile_pool(name="const", bufs=1) as constp, \
             tc.tile_pool(name="ats", bufs=2 * NCORES) as atp, \
             tc.tile_pool(name="ys", bufs=JT // 4) as yp, \
             tc.tile_pool(name="small", bufs=1) as smallp, \
             tc.tile_pool(name="outs", bufs=3) as outp, \
             tc.tile_pool(name="zacc", bufs=2, space="PSUM") as zps, \
             tc.tile_pool(name="rb", bufs=3, space="PSUM") as rbp, \
             tc.tile_pool(name="ups", bufs=3, space="PSUM") as ups:

            # ---- ncfw warmup: tiny collective queued behind the entry
            #      barrier so the first real AllGather starts on a warm ncfw ----
            dml = smallp.tile([1, 2], f32)
            nc.vector.memset(dml[:, :], 0.0)
            nc.scalar.dma_start(out=dumL[:, :], in_=dml[:, :])
            nc.gpsimd.collective_compute(
                "AllGather", mybir.AluOpType.bypass,
                replica_groups=[list(range(NCORES))],
                ins=[dumL[:, :]], outs=[dumA[:, :]],
            )
            # ---- small constants first (rowsums need ones1 immediately) ----
            ident = constp.tile([P, P], f32)
            nc.sync.dma_start(out=ident[:, :], in_=Idn[:, :])
            wt_sb = constp.tile([P, F], bf16)
            nc.sync.dma_start(out=wt_sb[:, :], in_=Wt[:, :])
            bb_sb = constp.tile([P, F], f32)
            nc.sync.dma_start(out=bb_sb[:, :], in_=Bb[:, :])
            ones1 = constp.tile([P, 1], bf16)
            nc.sync.dma_start(out=ones1[:, :], in_=On1[:, :])
            perm1 = constp.tile([JT // 2, JT // 2], f32)
            nc.sync.dma_start(out=perm1[:, :], in_=Pm1[:, :])
            perm2 = constp.tile([JT // 2, JT // 2], f32)
            nc.sync.dma_start(out=perm2[:, :], in_=Pm2[:, :])

            # ---- xt first (U matmuls fill the rs0 chain's DMA gaps),
            #      then A^T strip half0 groups, then half1 groups ----
            xt_sb = constp.tile([P, N], bf16)
            nc.sync.dma_start(out=xt_sb[:, :], in_=Xt[:, :])
            GW = NG * H  # 4096 cols per group
            atA, atB = [], []
            for g in range(JT // NG):
                t = atp.tile([P, GW], bf16, tag="at")
                nc.sync.dma_start(out=t[:, :], in_=A_t[:, g * GW:(g + 1) * GW])
                atA.append(t)
            for g in range(JT // NG):
                t = atp.tile([P, GW], bf16, tag="at")
                nc.sync.dma_start(out=t[:, :], in_=A_t[:, HB + g * GW:HB + (g + 1) * GW])
                atB.append(t)

            def atv(s, h):  # [128, 512] view of slot s, i-half h
                grp = (atA, atB)[h][s // NG]
                return grp[:, (s % NG) * H:(s % NG + 1) * H]

            # ---- rs0 chain (row sums, i<512): purely DMA-paced ----
            rs0 = rbp.tile([1, H], f32, tag="rb")
            for s in range(JT):
                nc.tensor.matmul(rs0[:, :], ones1[:, :], atv(s, 0),
                                 start=(s == 0), stop=(s == JT - 1))
            rs0_sb = smallp.tile([1, H], f32)
            nc.vector.tensor_copy(rs0_sb[:, :], rs0[:, :])
            nc.scalar.dma_start(out=degL1[:, :], in_=rs0_sb[:, :])
            nc.gpsimd.collective_compute(
                "AllGather", mybir.AluOpType.bypass,
                replica_groups=[list(range(NCORES))],
                ins=[degL1[:, :]], outs=[degA1[:, :]],
            )

            # ---- U = X @ W^T (batched: 4 N=128 matmuls per PSUM bank,
            #      one wide drain each) — fills the atB DMA window ----
            y4 = []
            for q in range(JT // 4):
                u_ps = ups.tile([P, 4 * F], f32, tag="u")
                for k in range(4):
                    nc.tensor.matmul(u_ps[:, k * F:(k + 1) * F],
                                     xt_sb[:, (4 * q + k) * P:(4 * q + k + 1) * P],
                                     wt_sb[:, :], start=True, stop=True)
                yt = yp.tile([P, 4 * F], bf16, tag="y")
                if q % 2 == 0:
                    nc.vector.tensor_copy(yt[:, :], u_ps[:, :])
                else:
                    nc.scalar.copy(yt[:, :], u_ps[:, :])
                y4.append(yt)
            y_t = [y4[s // 4][:, (s % 4) * F:(s % 4 + 1) * F] for s in range(JT)]
            # local U (slots 0..7 of rotated xt are this core's nodes), fp32
            yloc = smallp.tile([P, SR], f32)
            for q in range(2):
                u_ps = ups.tile([P, 4 * F], f32, tag="u")
                for k in range(4):
                    nc.tensor.matmul(u_ps[:, k * F:(k + 1) * F],
                                     xt_sb[:, (4 * q + k) * P:(4 * q + k + 1) * P],
                                     wt_sb[:, :], start=True, stop=True)
                nc.vector.tensor_copy(yloc[:, q * 4 * F:(q + 1) * 4 * F], u_ps[:, :])

            rs1 = rbp.tile([1, H], f32, tag="rb")
            for s in range(JT):
                nc.tensor.matmul(rs1[:, :], ones1[:, :], atv(s, 1),
                                 start=(s == 0), stop=(s == JT - 1))
            rs1_sb = smallp.tile([1, H], f32)
            nc.vector.tensor_copy(rs1_sb[:, :], rs1[:, :])
            nc.scalar.dma_start(out=degL2[:, :], in_=rs1_sb[:, :])
            nc.gpsimd.collective_compute(
                "AllGather", mybir.AluOpType.bypass,
                replica_groups=[list(range(NCORES))],
                ins=[degL2[:, :]], outs=[degA2[:, :]],
            )

            # ---- PE keep-warm fillers bridging rs1-end -> dinvT1-ready ----
            fill_ps = ups.tile([1, H], f32, tag="u")
            for k in range(FILLN):
                nc.tensor.matmul(fill_ps[:, :], ones1[:, :],
                                 atv(k % JT, 1), start=True, stop=True)
            dmr = smallp.tile([NCORES, 2], f32)
            nc.scalar.dma_start(out=dmr[:, :], in_=dumA[:, :])
            fz = smallp.tile([1, 1], f32)
            nc.vector.tensor_scalar_mul(fz[:, :], fill_ps[0:1, 0:1], 0.0)
            nc.vector.tensor_add(fz[:, :], fz[:, :], dmr[0:1, 0:1])

            def rsqrt_newton(dst, src, pool, shape):
                # dst = (src+1)^-1/2 with one Newton step (sqrt LUT refine)
                sq = pool.tile(shape, f32, tag="rn1")
                nc.scalar.activation(sq, src, AF.Sqrt, bias=1.0)
                r0 = pool.tile(shape, f32, tag="rn2")
                nc.vector.reciprocal(r0, sq)
                d1 = pool.tile(shape, f32, tag="rn3")
                nc.vector.tensor_scalar_add(d1, src, 1.0)
                tt = pool.tile(shape, f32, tag="rn4")
                nc.vector.tensor_mul(tt, r0, r0)
                nc.vector.tensor_mul(tt, tt, d1)
                nc.scalar.activation(tt, tt, AF.Copy, bias=1.5, scale=-0.5)
                nc.vector.tensor_mul(dst, r0, tt)

            dpad = smallp.tile([P, P], f32)
            nc.vector.memset(dpad[:, :], 0.0)

            def dinv_chain(degA, perm_sb, name):
                # gathered degrees -> rotated slot order -> rsqrt -> [128,32]
                dsb = smallp.tile([JT // 2, P], f32, tag=name + "d")
                nc.sync.dma_start(out=dsb[:, :], in_=degA[:, :])
                drp = rbp.tile([JT // 2, P], f32, tag="rb")
                nc.tensor.matmul(drp[:, :], perm_sb[:, :], dsb[:, :],
                                 start=True, stop=True)
                dgr = smallp.tile([JT // 2, P], f32, tag=name + "r")
                nc.vector.tensor_copy(dgr[:, :], drp[:, :])
                dinv = smallp.tile([JT // 2, P], f32, tag=name + "i")
                rsqrt_newton(dinv[:, :], dgr[:, :], smallp, [JT // 2, P])
                nc.vector.tensor_copy(dpad[0:JT // 2, :], dinv[:, :])
                dtp = rbp.tile([P, P], f32, tag="rb")
                nc.tensor.transpose(dtp[:, :], dpad[:, :], ident[:, :])
                dT = smallp.tile([P, JT // 2], f32, tag=name + "t")
                nc.vector.tensor_copy(dT[:, :], dtp[:, 0:JT // 2])
                return dT

            dinvT1 = dinv_chain(degA1, perm1, "g1")

            def dcol(s):  # [128,1] dinv column for slot s
                if s % 8 < 4:
                    return dinvT1[:, (s // 8) * 4 + s % 8:(s // 8) * 4 + s % 8 + 1]
                return dinvT2[:, (s // 8) * 4 + s % 8 - 4:(s // 8) * 4 + s % 8 - 3]

            # ---- big matmul Z^T, AG1-covered slots ----
            z0 = zps.tile([P, H], f32, tag="z")
            z1 = zps.tile([P, H], f32, tag="z")
            for k, s in enumerate(S1):
                nc.vector.tensor_scalar_mul(y_t[s], y_t[s], dcol(s))
                nc.tensor.matmul(z0[:, :], y_t[s], atv(s, 0),
                                 start=(k == 0), stop=False)
                nc.tensor.matmul(z1[:, :], y_t[s], atv(s, 1),
                                 start=(k == 0), stop=False)

            dinvT2 = dinv_chain(degA2, perm2, "g2")

            # ---- big matmul Z^T, AG2-covered slots ----
            for k, s in enumerate(S2):
                nc.vector.tensor_scalar_mul(y_t[s], y_t[s], dcol(s))
                nc.tensor.matmul(z0[:, :], y_t[s][:, :], atv(s, 0),
                                 start=False, stop=(k == len(S2) - 1))
                nc.tensor.matmul(z1[:, :], y_t[s][:, :], atv(s, 1),
                                 start=False, stop=(k == len(S2) - 1))

            # yq = d_i^-1/2 * Y_local + b = d^-1 U_loc + b (free time, DVE)
            for it in range(IT):
                nc.vector.tensor_scalar_mul(
                    yloc[:, it * F:(it + 1) * F], yloc[:, it * F:(it + 1) * F],
                    dcol(it))
                nc.vector.tensor_scalar_mul(
                    yloc[:, it * F:(it + 1) * F], yloc[:, it * F:(it + 1) * F],
                    dcol(it))
                nc.vector.tensor_add(
                    yloc[:, it * F:(it + 1) * F], yloc[:, it * F:(it + 1) * F],
                    bb_sb[:, :])

            # ---- epilogue: un-transpose Z^T, + Y_local, * d_i^-1/2, + b ----
            ztsb = smallp.tile([P, SR], f32)
            nc.vector.tensor_copy(ztsb[:, 0:H], z0[:, :])
            nc.vector.tensor_copy(ztsb[:, H:SR], z1[:, :])
            # consume the filler sink (exact zero) so nothing is dead code
            nc.vector.tensor_add(ztsb[0:1, 0:1], ztsb[0:1, 0:1], fz[:, :])
            for it in range(IT):
                tp = rbp.tile([P, P], f32, tag="rb")
                nc.tensor.transpose(tp[:, :], ztsb[:, it * P:(it + 1) * P],
                                    ident[:, :])
                o = outp.tile([P, F], f32, tag="o")
                nc.vector.tensor_scalar_mul(o[:, :], tp[:, :], dcol(it))
                nc.vector.tensor_add(o[:, :], o[:, :],
                                     yloc[:, it * F:(it + 1) * F])
                nc.sync.dma_start(out=out[it * P:(it + 1) * P, :], in_=o[:, :])

    return nc


_NO_SPLIT_TYPES = ("InstEventSemaphore", "InstSemaphore", "InstTrigger")


def _split_drain_waits(nc, max_waits=1):
    """This walrus build only encodes one sem-wait per instruction; hoist
    extras onto preceding same-engine NOPs (monotonic sems => equivalent)."""
    import concourse.mybir as mybir
    for fn in nc.m.functions:
        for blk in fn.blocks:
            newlist = []
            for ins in blk.instructions:
                si = getattr(ins, "sync_info", None)
                tname = type(ins).__name__
                if si is not None and si.on_wait and len(si.on_wait) > max_waits \
                        and not any(tname.startswith(t) for t in _NO_SPLIT_TYPES):
                    waits = list(si.on_wait)
                    for j, w in enumerate(waits[max_waits:]):
                        newlist.append(mybir.InstNoOp(
                            name=f"{ins.name}-w{j}", engine=ins.engine,
                            ins=[], outs=[],
                            sync_info=mybir.SyncInfo(on_wait=[w], on_update=[]),
                        ))
                    si.on_wait = waits[:max_waits]
                newlist.append(ins)
            blk.instructions[:] = newlist


def _get_nc():
    if "nc" not in _CACHE:
        nc = _build_nc()
        _split_drain_waits(nc)
        _CACHE["nc"] = nc
    return _CACHE["nc"]


def _make_in_maps(X, A, W, b):
    bf16 = ml_dtypes.bfloat16
    X = np.ascontiguousarray(np.asarray(X, dtype=np.float32))
    A = np.ascontiguousarray(np.asarray(A, dtype=np.float32))
    W = np.ascontiguousarray(np.asarray(W, dtype=np.float32))
    b = np.ascontiguousarray(np.asarray(b, dtype=np.float32))
    At_bf = np.asarray(A.T, dtype=bf16)          # [N, N] bf16, column c-strips
    Xt_bf = np.ascontiguousarray(X.T).astype(bf16)
    Wt_bf = np.ascontiguousarray(W.T).astype(bf16)
    Bb = np.ascontiguousarray(np.tile(b[None, :], (P, 1)))
    Idn = np.eye(P, dtype=np.float32)
    On1 = np.ones((P, 1), dtype=bf16)
    HJ = JT // 2

    def permmat(c, slots):
        # gathered-degree row of global tile t=(c*8+s)%64 is (t//8)*4 + t%8%4
        pm = np.zeros((HJ, HJ), dtype=np.float32)
        for q, s in enumerate(slots):
            t = (c * IT + s) % JT
            pm[(t // 8) * 4 + (t % 8) % 4, q] = 1.0
        return pm

    in_maps = []
    for c in range(NCORES):
        at_strip = At_bf[:, c * SR:(c + 1) * SR]           # [8192, 1024]
        at_rot = np.roll(at_strip, -c * SR, axis=0)
        r = at_rot.reshape(JT, P, SR)
        at_h = np.empty((P, JT * SR), dtype=bf16)
        at_h[:, :JT * H] = r[:, :, :H].transpose(1, 0, 2).reshape(P, JT * H)
        at_h[:, JT * H:] = r[:, :, H:].transpose(1, 0, 2).reshape(P, JT * H)
        xt_rot = np.ascontiguousarray(np.roll(Xt_bf, -c * SR, axis=1))
        in_maps.append({
            "at": at_h,
            "xt": xt_rot,
            "wt": Wt_bf,
            "bb": Bb,
            "ident": Idn,
            "ones1": On1,
            "perm1": permmat(c, S1),
            "perm2": permmat(c, S2),
        })
    return in_maps


def _install_ntff_hook():
    """This image's antenv lacks axon_hooks; synthesize it so trace=True
    can reach the terminal's NTFF capture via the libaxon ctypes hook."""
    import sys
    import types
    if "antenv.axon_hooks" in sys.modules:
        return
    try:
        from trn_agent_boot.trn_boot import _ntff_profile_via_ctypes
        hook = _ntff_profile_via_ctypes("/opt/axon/libaxon_pjrt.so")
    except Exception:
        hook = None
    mod = types.ModuleType("antenv.axon_hooks")
    mod._hook = hook
    mod.get_axon_ntff_profile_hook = lambda: mod._hook

    def _set(h):
        mod._hook = h
    mod.set_axon_ntff_profile_hook = _set
    sys.modules["antenv.axon_hooks"] = mod
    import antenv
    antenv.axon_hooks = mod
    # the artifact upload needs a bucket this sandbox doesn't have
    import concourse.bass_utils as bu
    bu.upload_artifacts = lambda tmpdir: f"local:{tmpdir}"


def run(X, A, W, b, trace=False, **trace_kwargs):
    """Run on hardware; returns (output, BassKernelResults)."""
    from concourse.bass_utils import run_bass_kernel_spmd
    if trace:
        _install_ntff_hook()
    nc = _get_nc()
    in_maps = _make_in_maps(X, A, W, b)
    res = run_bass_kernel_spmd(nc, in_maps, list(range(NCORES)),
                               trace=trace, **trace_kwargs)
    outs = [np.asarray(res.results[c]["out"], dtype=np.float32)
            for c in range(NCORES)]
    return np.concatenate(outs, axis=0), res


def kernel(X, A, W, b):
    out, _ = run(X, A, W, b, trace=False)
    return out


# revision 21
# speedup vs baseline: 1.2541x; 1.2541x over previous
"""GCN layer kernel for 8 trn2 NeuronCores (SPMD, single launch).

Math:  out = D^-1/2 (A+I) D^-1/2 X W^T + b
Identity: the dense layer commutes with the diagonal scalings:
    out = D^-1/2 (A+I) D^-1/2 (X W^T) + b
so U = X@W^T (tiny) is computed on-chip, then one big matmul A @ (d^-1/2*U).

Sharding: row-shard A (1024 rows/core).  Each core receives its strip
PRE-TRANSPOSED to A^T layout and cast to bf16 on the host (16.8MB/core,
half the fp32 bytes; the transposed layout is what the PE contraction
needs, eliminating on-device PE transposes).  The strip is shipped in
SBUF order ([128, 65536], half-i major) so DMA runs as 16 x 1MB
transfers with 8KB contiguous per partition line.

The per-core j-axis (contraction axis) is ROTATED so each core's own
1024 nodes occupy j-tile slots 0..7: slot s holds global tile
(c*8+s)%64.  With this, "local vs remote" slot sets are uniform across
the shared SPMD program (slot s covers AG-half s%8//4 on every core).

Degree exchange is a SPLIT AllGather: rows i<512 of every core finish
DMA first, their row sums AllGather (AG1) while the second half still
streams; AG2 goes out at stream end and overlaps with the first half
of the big matmul.  Between the two matmul halves, dinv-gated filler
matmuls keep the PE HAM clock at 8/8.

Per core:
  - stream A^T strip; row sums accumulate on PE via a ones-column
    stationary matmul (one [1,512] PSUM chain per i-half).
  - U = X@W^T for all 8192 nodes (64 small matmuls, overlaps DMA).
  - AG1/AG2 local row sums -> full degree; per-core permutation
    matmuls map gathered degrees to rotated slot order; rsqrt+Newton.
  - Z^T accumulates in 2 PSUM banks: for each slot s,
    zacc_h += Y_s[j,f].T @ At_s[j,i_h]   (Y stationary, N=512 moving)
  - epilogue: 8 PE transposes of Z^T -> natural Z, + Y_local,
    * d_i^-1/2, + b, DMA out.
"""

import numpy as np
import ml_dtypes

N = 8192          # nodes
F = 128           # in/out feature dim
NCORES = 8
SR = N // NCORES  # strip rows per core = 1024
P = 128           # partitions / tile edge
IT = SR // P      # 8 row tiles per strip
JT = N // P       # 64 contraction tiles
H = 512           # psum-bank half of a 1024-wide row
NG = 8            # slots per DMA group (1MB transfers)
FILLN = 90        # PE keep-warm fillers between matmul halves

# slots whose global tile lands in AG-half 1 vs 2 (uniform across cores:
# slot s holds global tile (c*8+s)%64, whose i-half is (s%8)//4)
S1 = [s for s in range(JT) if s % 8 < 4]
S2 = [s for s in range(JT) if s % 8 >= 4]

_CACHE = {}


def _build_nc():
    import concourse.mybir as mybir
    from concourse import bass
    from concourse.tile import TileContext

    f32 = mybir.dt.float32
    bf16 = mybir.dt.bfloat16
    AF = mybir.ActivationFunctionType

    nc = bass.Bass(num_devices=NCORES)

    # at layout: [:, 0:32768] half0 = cols s*512+i (i<512) of slot s;
    #            [:, 32768:]  half1 = cols 32768+s*512+(i-512)
    A_t = nc.declare_dram_parameter("at", [P, JT * SR], bf16, False)
    Xt = nc.declare_dram_parameter("xt", [P, N], bf16, False)      # rotated X^T
    Wt = nc.declare_dram_parameter("wt", [P, F], bf16, False)      # W^T
    Bb = nc.declare_dram_parameter("bb", [P, F], f32, False)       # bias bcast
    Idn = nc.declare_dram_parameter("ident", [P, P], f32, False)
    On1 = nc.declare_dram_parameter("ones1", [P, 1], bf16, False)
    Pm1 = nc.declare_dram_parameter("perm1", [JT // 2, JT // 2], f32, False)
    Pm2 = nc.declare_dram_parameter("perm2", [JT // 2, JT // 2], f32, False)
    out = nc.declare_dram_parameter("out", [SR, F], f32, True)

    dumL = nc.dram_tensor("dum_loc", [1, 2], f32)
    dumA = nc.dram_tensor("dum_all", [NCORES, 2], f32, addr_space="Shared")
    degL1 = nc.dram_tensor("deg_loc1", [1, H], f32)
    degL2 = nc.dram_tensor("deg_loc2", [1, H], f32)
    degA1 = nc.dram_tensor("deg_all1", [JT // 2, P], f32, addr_space="Shared")
    degA2 = nc.dram_tensor("deg_all2", [JT // 2, P], f32, addr_space="Shared")

    HB = JT * H  # 32768, half-block width in the at layout

    with TileContext(nc) as tc:
        with tc.tile_pool(name="const", bufs=1) as constp, \
             tc.tile_pool(name="ats", bufs=2 * NCORES) as atp, \
             tc.tile_pool(name="ys", bufs=JT // 4) as yp, \
             tc.tile_pool(name="small", bufs=1) as smallp, \
             tc.tile_pool(name="outs", bufs=3) as outp, \
             tc.tile_pool(name="zacc", bufs=2, space="PSUM") as zps, \
             tc.tile_pool(name="rb", bufs=3, space="PSUM") as rbp, \
             tc.tile_pool(name="ups", bufs=3, space="PSUM") as ups:

            # ---- ncfw warmup: tiny collective queued behind the entry
            #      barrier so the first real AllGather starts on a warm ncfw ----
            dml = smallp.tile([1, 2], f32)
            nc.vector.memset(dml[:, :], 0.0)
            nc.scalar.dma_start(out=dumL[:, :], in_=dml[:, :])
            nc.gpsimd.collective_compute(
                "AllGather", mybir.AluOpType.bypass,
                replica_groups=[list(range(NCORES))],
                ins=[dumL[:, :]], outs=[dumA[:, :]],
            )
            # ---- small constants first (rowsums need ones1 immediately) ----
            ident = constp.tile([P, P], f32)
            nc.sync.dma_start(out=ident[:, :], in_=Idn[:, :])
            wt_sb = constp.tile([P, F], bf16)
            nc.sync.dma_start(out=wt_sb[:, :], in_=Wt[:, :])
            bb_sb = constp.tile([P, F], f32)
            nc.sync.dma_start(out=bb_sb[:, :], in_=Bb[:, :])
            ones1 = constp.tile([P, 1], bf16)
            nc.sync.dma_start(out=ones1[:, :], in_=On1[:, :])
            perm1 = constp.tile([JT // 2, JT // 2], f32)
            nc.sync.dma_start(out=perm1[:, :], in_=Pm1[:, :])
            perm2 = constp.tile([JT // 2, JT // 2], f32)
            nc.sync.dma_start(out=perm2[:, :], in_=Pm2[:, :])

            # ---- xt first (U matmuls fill the rs0 chain's DMA gaps),
            #      then A^T strip half0 groups, then half1 groups ----
            xt_sb = constp.tile([P, N], bf16)
            nc.sync.dma_start(out=xt_sb[:, :], in_=Xt[:, :])
            GW = NG * H  # 4096 cols per group
            atA, atB = [], []
            for g in range(JT // NG):
                t = atp.tile([P, GW], bf16, tag="at")
                nc.sync.dma_start(out=t[:, :], in_=A_t[:, g * GW:(g + 1) * GW])
                atA.append(t)
            for g in range(JT // NG):
                t = atp.tile([P, GW], bf16, tag="at")
                nc.sync.dma_start(out=t[:, :], in_=A_t[:, HB + g * GW:HB + (g + 1) * GW])
                atB.append(t)

            def atv(s, h):  # [128, 512] view of slot s, i-half h
                grp = (atA, atB)[h][s // NG]
                return grp[:, (s % NG) * H:(s % NG + 1) * H]

            # ---- rs0 chain (row sums, i<512): purely DMA-paced ----
            rs0 = rbp.tile([1, H], f32, tag="rb")
            for s in range(JT):
                nc.tensor.matmul(rs0[:, :], ones1[:, :], atv(s, 0),
                                 start=(s == 0), stop=(s == JT - 1))
            rs0_sb = smallp.tile([1, H], f32)
            nc.vector.tensor_copy(rs0_sb[:, :], rs0[:, :])
            nc.scalar.dma_start(out=degL1[:, :], in_=rs0_sb[:, :])
            nc.gpsimd.collective_compute(
                "AllGather", mybir.AluOpType.bypass,
                replica_groups=[list(range(NCORES))],
                ins=[degL1[:, :]], outs=[degA1[:, :]],
            )

            # ---- U = X @ W^T (batched: 4 N=128 matmuls per PSUM bank,
            #      one wide drain each) — fills the atB DMA window ----
            y4 = []
            for q in range(JT // 4):
                u_ps = ups.tile([P, 4 * F], f32, tag="u")
                for k in range(4):
                    nc.tensor.matmul(u_ps[:, k * F:(k + 1) * F],
                                     xt_sb[:, (4 * q + k) * P:(4 * q + k + 1) * P],
                                     wt_sb[:, :], start=True, stop=True)
                yt = yp.tile([P, 4 * F], bf16, tag="y")
                if q % 2 == 0:
                    nc.vector.tensor_copy(yt[:, :], u_ps[:, :])
                else:
                    nc.scalar.copy(yt[:, :], u_ps[:, :])
                y4.append(yt)
            y_t = [y4[s // 4][:, (s % 4) * F:(s % 4 + 1) * F] for s in range(JT)]
            # local U (slots 0..7 of rotated xt are this core's nodes), fp32
            yloc = smallp.tile([P, SR], f32)
            for q in range(2):
                u_ps = ups.tile([P, 4 * F], f32, tag="u")
                for k in range(4):
                    nc.tensor.matmul(u_ps[:, k * F:(k + 1) * F],
                                     xt_sb[:, (4 * q + k) * P:(4 * q + k + 1) * P],
                                     wt_sb[:, :], start=True, stop=True)
                nc.vector.tensor_copy(yloc[:, q * 4 * F:(q + 1) * 4 * F], u_ps[:, :])

            rs1 = rbp.tile([1, H], f32, tag="rb")
            for s in range(JT):
                nc.tensor.matmul(rs1[:, :], ones1[:, :], atv(s, 1),
                                 start=(s == 0), stop=(s == JT - 1))
            rs1_sb = smallp.tile([1, H], f32)
            nc.vector.tensor_copy(rs1_sb[:, :], rs1[:, :])
            nc.scalar.dma_start(out=degL2[:, :], in_=rs1_sb[:, :])
            nc.gpsimd.collective_compute(
                "AllGather", mybir.AluOpType.bypass,
                replica_groups=[list(range(NCORES))],
                ins=[degL2[:, :]], outs=[degA2[:, :]],
            )

            # ---- PE keep-warm fillers bridging rs1-end -> dinvT1-ready ----
            fill_ps = ups.tile([1, H], f32, tag="u")
            for k in range(FILLN):
                nc.tensor.matmul(fill_ps[:, :], ones1[:, :],
                                 atv(k % JT, 1), start=True, stop=True)
            dmr = smallp.tile([NCORES, 2], f32)
            nc.scalar.dma_start(out=dmr[:, :], in_=dumA[:, :])
            fz = smallp.tile([1, 1], f32)
            nc.vector.tensor_scalar_mul(fz[:, :], fill_ps[0:1, 0:1], 0.0)
            nc.vector.tensor_add(fz[:, :], fz[:, :], dmr[0:1, 0:1])

            def rsqrt_newton(dst, src, pool, shape):
                # dst = (src+1)^-1/2 with one Newton step (sqrt LUT refine)
                sq = pool.tile(shape, f32, tag="rn1")
                nc.scalar.activation(sq, src, AF.Sqrt, bias=1.0)
                r0 = pool.tile(shape, f32, tag="rn2")
                nc.vector.reciprocal(r0, sq)
                d1 = pool.tile(shape, f32, tag="rn3")
                nc.vector.tensor_scalar_add(d1, src, 1.0)
                tt = pool.tile(shape, f32, tag="rn4")
                nc.vector.tensor_mul(tt, r0, r0)
                nc.vector.tensor_mul(tt, tt, d1)
                nc.scalar.activation(tt, tt, AF.Copy, bias=1.5, scale=-0.5)
                nc.vector.tensor_mul(dst, r0, tt)

            dpad = smallp.tile([P, P], f32)
            nc.vector.memset(dpad[:, :], 0.0)

            def dinv_chain(degA, perm_sb, name):
                # gathered degrees -> rotated slot order -> rsqrt -> [128,32]
                dsb = smallp.tile([JT // 2, P], f32, tag=name + "d")
                nc.sync.dma_start(out=dsb[:, :], in_=degA[:, :])
                drp = rbp.tile([JT // 2, P], f32, tag="rb")
                nc.tensor.matmul(drp[:, :], perm_sb[:, :], dsb[:, :],
                                 start=True, stop=True)
                dgr = smallp.tile([JT // 2, P], f32, tag=name + "r")
                nc.vector.tensor_copy(dgr[:, :], drp[:, :])
                dinv = smallp.tile([JT // 2, P], f32, tag=name + "i")
                rsqrt_newton(dinv[:, :], dgr[:, :], smallp, [JT // 2, P])
                nc.vector.tensor_copy(dpad[0:JT // 2, :], dinv[:, :])
                dtp = rbp.tile([P, P], f32, tag="rb")
                nc.tensor.transpose(dtp[:, :], dpad[:, :], ident[:, :])
                dT = smallp.tile([P, JT // 2], f32, tag=name + "t")
                nc.vector.tensor_copy(dT[:, :], dtp[:, 0:JT // 2])
                return dT

            dinvT1 = dinv_chain(degA1, perm1, "g1")

            def dcol(s):  # [128,1] dinv column for slot s
                if s % 8 < 4:
                    return dinvT1[:, (s // 8) * 4 + s % 8:(s // 8) * 4 + s % 8 + 1]
                return dinvT2[:, (s // 8) * 4 + s % 8 - 4:(s // 8) * 4 + s % 8 - 3]

            # ---- big matmul Z^T, AG1-covered slots ----
            z0 = zps.tile([P, H], f32, tag="z")
            z1 = zps.tile([P, H], f32, tag="z")
            for k, s in enumerate(S1):
                nc.vector.tensor_scalar_mul(y_t[s], y_t[s], dcol(s))
                nc.tensor.matmul(z0[:, :], y_t[s], atv(s, 0),
                                 start=(k == 0), stop=False)
                nc.tensor.matmul(z1[:, :], y_t[s], atv(s, 1),
                                 start=(k == 0), stop=False)

            dinvT2 = dinv_chain(degA2, perm2, "g2")

            # ---- big matmul Z^T, AG2-covered slots ----
            for k, s in enumerate(S2):
                nc.vector.tensor_scalar_mul(y_t[s], y_t[s], dcol(s))
                nc.tensor.matmul(z0[:, :], y_t[s][:, :], atv(s, 0),
                                 start=False, stop=(k == len(S2) - 1))
                nc.tensor.matmul(z1[:, :], y_t[s][:, :], atv(s, 1),
                                 start=False, stop=(k == len(S2) - 1))

            # yq = d_i^-1/2 * Y_local + b = d^-1 U_loc + b (free time, DVE)
            for it in range(IT):
                nc.vector.tensor_scalar_mul(
                    yloc[:, it * F:(it + 1) * F], yloc[:, it * F:(it + 1) * F],
                    dcol(it))
                nc.vector.tensor_scalar_mul(
                    yloc[:, it * F:(it + 1) * F], yloc[:, it * F:(it + 1) * F],
                    dcol(it))
                nc.vector.tensor_add(
                    yloc[:, it * F:(it + 1) * F], yloc[:, it * F:(it + 1) * F],
                    bb_sb[:, :])

            # ---- epilogue: un-transpose Z^T, + Y_local, * d_i^-1/2, + b ----
            ztsb = smallp.tile([P, SR], f32)
            nc.vector.tensor_copy(ztsb[:, 0:H], z0[:, :])
            nc.vector.tensor_copy(ztsb[:, H:SR], z1[:, :])
            # consume the filler sink (exact zero) so nothing is dead code
            nc.vector.tensor_add(ztsb[0:1, 0:1], ztsb[0:1, 0:1], fz[:, :])
            for it in range(IT):
                tp = rbp.tile([P, P], f32, tag="rb")
                nc.tensor.transpose(tp[:, :], ztsb[:, it * P:(it + 1) * P],
                                    ident[:, :])
                o = outp.tile([P, F], f32, tag="o")
                nc.vector.tensor_scalar_mul(o[:, :], tp[:, :], dcol(it))
                nc.vector.tensor_add(o[:, :], o[:, :],
                                     yloc[:, it * F:(it + 1) * F])
                nc.sync.dma_start(out=out[it * P:(it + 1) * P, :], in_=o[:, :])

    return nc


_NO_SPLIT_TYPES = ("InstEventSemaphore", "InstSemaphore", "InstTrigger")


def _split_drain_waits(nc, max_waits=1):
    """This walrus build only encodes one sem-wait per instruction; hoist
    extras onto preceding same-engine NOPs (monotonic sems => equivalent)."""
    import concourse.mybir as mybir
    for fn in nc.m.functions:
        for blk in fn.blocks:
            newlist = []
            for ins in blk.instructions:
                si = getattr(ins, "sync_info", None)
                tname = type(ins).__name__
                if si is not None and si.on_wait and len(si.on_wait) > max_waits \
                        and not any(tname.startswith(t) for t in _NO_SPLIT_TYPES):
                    waits = list(si.on_wait)
                    for j, w in enumerate(waits[max_waits:]):
                        newlist.append(mybir.InstNoOp(
                            name=f"{ins.name}-w{j}", engine=ins.engine,
                            ins=[], outs=[],
                            sync_info=mybir.SyncInfo(on_wait=[w], on_update=[]),
                        ))
                    si.on_wait = waits[:max_waits]
                newlist.append(ins)
            blk.instructions[:] = newlist


def _get_nc():
    if "nc" not in _CACHE:
        nc = _build_nc()
        _split_drain_waits(nc)
        _CACHE["nc"] = nc
    return _CACHE["nc"]


def _make_in_maps(X, A, W, b):
    bf16 = ml_dtypes.bfloat16
    X = np.ascontiguousarray(np.asarray(X, dtype=np.float32))
    A = np.ascontiguousarray(np.asarray(A, dtype=np.float32))
    W = np.ascontiguousarray(np.asarray(W, dtype=np.float32))
    b = np.ascontiguousarray(np.asarray(b, dtype=np.float32))
    At_bf = np.asarray(A.T, dtype=bf16)          # [N, N] bf16, column c-strips
    Xt_bf = np.ascontiguousarray(X.T).astype(bf16)
    Wt_bf = np.ascontiguousarray(W.T).astype(bf16)
    Bb = np.ascontiguousarray(np.tile(b[None, :], (P, 1)))
    Idn = np.eye(P, dtype=np.float32)
    On1 = np.ones((P, 1), dtype=bf16)
    HJ = JT // 2

    def permmat(c, slots):
        # gathered-degree row of global tile t=(c*8+s)%64 is (t//8)*4 + t%8%4
        pm = np.zeros((HJ, HJ), dtype=np.float32)
        for q, s in enumerate(slots):
            t = (c * IT + s) % JT
            pm[(t // 8) * 4 + (t % 8) % 4, q] = 1.0
        return pm

    in_maps = []
    for c in range(NCORES):
        at_strip = At_bf[:, c * SR:(c + 1) * SR]           # [8192, 1024]
        at_rot = np.roll(at_strip, -c * SR, axis=0)
        r = at_rot.reshape(JT, P, SR)
        at_h = np.empty((P, JT * SR), dtype=bf16)
        at_h[:, :JT * H] = r[:, :, :H].transpose(1, 0, 2).reshape(P, JT * H)
        at_h[:, JT * H:] = r[:, :, H:].transpose(1, 0, 2).reshape(P, JT * H)
        xt_rot = np.ascontiguousarray(np.roll(Xt_bf, -c * SR, axis=1))
        in_maps.append({
            "at": at_h,
            "xt": xt_rot,
            "wt": Wt_bf,
            "bb": Bb,
            "ident": Idn,
            "ones1": On1,
            "perm1": permmat(c, S1),
            "perm2": permmat(c, S2),
        })
    return in_maps


def _install_ntff_hook():
    """This image's antenv lacks axon_hooks; synthesize it so trace=True
    can reach the terminal's NTFF capture via the libaxon ctypes hook."""
    import sys
    import types
    if "antenv.axon_hooks" in sys.modules:
        return
    try:
        from trn_agent_boot.trn_boot import _ntff_profile_via_ctypes
        hook = _ntff_profile_via_ctypes("/opt/axon/libaxon_pjrt.so")
    except Exception:
        hook = None
    mod = types.ModuleType("antenv.axon_hooks")
    mod._hook = hook
    mod.get_axon_ntff_profile_hook = lambda: mod._hook

    def _set(h):
        mod._hook = h
    mod.set_axon_ntff_profile_hook = _set
    sys.modules["antenv.axon_hooks"] = mod
    import antenv
    antenv.axon_hooks = mod
    # the artifact upload needs a bucket this sandbox doesn't have
    import concourse.bass_utils as bu
    bu.upload_artifacts = lambda tmpdir: f"local:{tmpdir}"


def run(X, A, W, b, trace=False, **trace_kwargs):
    """Run on hardware; returns (output, BassKernelResults)."""
    from concourse.bass_utils import run_bass_kernel_spmd
    if trace:
        _install_ntff_hook()
    nc = _get_nc()
    in_maps = _make_in_maps(X, A, W, b)
    res = run_bass_kernel_spmd(nc, in_maps, list(range(NCORES)),
                               trace=trace, **trace_kwargs)
    outs = [np.asarray(res.results[c]["out"], dtype=np.float32)
            for c in range(NCORES)]
    return np.concatenate(outs, axis=0), res


def kernel(X, A, W, b):
    out, _ = run(X, A, W, b, trace=False)
    return out
